# revision 7
# baseline (speedup 1.0000x reference)
"""Trainium2 Bass kernel for a 4-layer GPT classifier (CMGPTClassifier).

Strategy: data-parallel over batch — each of the 8 NeuronCores runs the full
model on one sequence. All activations stay resident in SBUF in a
"layout B" = [feature-on-partitions, tokens-in-free] layout; weights stream
from HBM (f32) and are cast to bf16 on-chip; matmuls run in bf16 with f32
PSUM accumulation.

Model (per core): S=1024 tokens, D=768, H=12 heads (HS=64), FF=3072, L=4
layers, 16 classes. h = tok_emb[x] + pos_emb; per layer:
  xn  = LN1(h);  q,k,v per head;  att = softmax(q k^T / sqrt(D)) v
  h  += concat(att) @ Wo + bo
  xn2 = LN2(h);  h += relu(xn2 @ W1 + b1) @ W2 + b2
logits = relu(LNf(h)[last] @ cW1 + cb1) @ cW2 + cb2

Implementation notes:
 - LN over the partition (feature) axis uses a ones[128,128] stationary
   matmul to produce per-token sums broadcast across partitions, so all the
   stat math runs as full-width [128, 512] vector ops.
 - Attention works in the transposed layout: scores_T[key, query] tiles;
   exp runs on ScalarE straight out of PSUM with the attention scale and the
   key-mask bias folded in.  The softmax denominator comes for free from a
   ones-column appended to V (lhsT [128, 65]); normalization multiplies by a
   PE-broadcast reciprocal row.
"""

from contextlib import ExitStack

import numpy as np

import concourse.bacc as bacc
import concourse.bass as bass
import concourse.mybir as mybir
import concourse.tile as tile
from concourse.bass_utils import run_bass_kernel_spmd
from concourse.masks import make_identity

F32 = mybir.dt.float32
BF16 = mybir.dt.bfloat16
I32 = mybir.dt.int32
AF = mybir.ActivationFunctionType
ALU = mybir.AluOpType

P = 128


def build_nc(S=1024, L=4, H=12, D=768, FF=3072, V=32000, NCLS=16):
    HS = D // H
    KD = D // P          # 6 feature tiles
    KF = FF // P         # 24 ff tiles
    NT = S // P          # token tiles
    QBS = min(512, S)    # token block for matmul free dim
    NQ = S // QBS
    HP = H // 2          # head pairs
    SCALE = float(D) ** -0.5

    nc = bacc.Bacc("TRN2", target_bir_lowering=False)

    ids_d = nc.dram_tensor("ids", [S], I32, kind="ExternalInput")
    mask_d = nc.dram_tensor("mask", [S], I32, kind="ExternalInput")
    temb_d = nc.dram_tensor("tok_emb", [V, D], F32, kind="ExternalInput")
    pemb_d = nc.dram_tensor("pos_emb", [S, D], F32, kind="ExternalInput")
    wq_d = nc.dram_tensor("Wq", [L, H, D, HS], F32, kind="ExternalInput")
    wk_d = nc.dram_tensor("Wk", [L, H, D, HS], F32, kind="ExternalInput")
    wv_d = nc.dram_tensor("Wv", [L, H, D, HS], F32, kind="ExternalInput")
    wo_d = nc.dram_tensor("Wo", [L, D, D], F32, kind="ExternalInput")
    bo_d = nc.dram_tensor("bo", [L, D], F32, kind="ExternalInput")
    ln1w_d = nc.dram_tensor("ln1_w", [L, D], F32, kind="ExternalInput")
    ln1b_d = nc.dram_tensor("ln1_b", [L, D], F32, kind="ExternalInput")
    ln2w_d = nc.dram_tensor("ln2_w", [L, D], F32, kind="ExternalInput")
    ln2b_d = nc.dram_tensor("ln2_b", [L, D], F32, kind="ExternalInput")
    w1_d = nc.dram_tensor("W1", [L, D, FF], F32, kind="ExternalInput")
    b1_d = nc.dram_tensor("b1", [L, FF], F32, kind="ExternalInput")
    w2_d = nc.dram_tensor("W2", [L, FF, D], F32, kind="ExternalInput")
    b2_d = nc.dram_tensor("b2", [L, D], F32, kind="ExternalInput")
    lnfw_d = nc.dram_tensor("lnf_w", [D], F32, kind="ExternalInput")
    lnfb_d = nc.dram_tensor("lnf_b", [D], F32, kind="ExternalInput")
    cw1_d = nc.dram_tensor("cW1", [D, FF], F32, kind="ExternalInput")
    cb1_d = nc.dram_tensor("cb1", [FF], F32, kind="ExternalInput")
    cw2_d = nc.dram_tensor("cW2", [FF, NCLS], F32, kind="ExternalInput")
    cb2_d = nc.dram_tensor("cb2", [NCLS], F32, kind="ExternalInput")
    out_d = nc.dram_tensor("out", [1, NCLS], F32, kind="ExternalOutput")

    with tile.TileContext(nc) as tc, ExitStack() as ctx:
        consts = ctx.enter_context(tc.tile_pool(name="consts", bufs=1))
        ones_bf = consts.tile([P, P], BF16, tag="ones")
        nc.vector.memset(ones_bf[:], 1.0)
        ident = consts.tile([P, P], F32, tag="ident")
        make_identity(nc, ident[:])
        eps_col = consts.tile([P, 1], F32, tag="eps")
        nc.vector.memset(eps_col[:], 1e-5)

        ids_sb = consts.tile([P, NT], I32, tag="ids")
        nc.sync.dma_start(out=ids_sb[:], in_=ids_d[:].rearrange("(t p) -> p t", p=P))
        mask_sb = consts.tile([P, NT], I32, tag="mask")
        nc.sync.dma_start(out=mask_sb[:], in_=mask_d[:].rearrange("(t p) -> p t", p=P))
        maskf = consts.tile([P, NT], F32, tag="maskf")
        nc.vector.tensor_copy(out=maskf[:], in_=mask_sb[:])
        mbias = consts.tile([P, NT], F32, tag="mbias")
        nc.vector.tensor_scalar(
            out=mbias[:], in0=maskf[:], scalar1=1.0, scalar2=30.0,
            op0=ALU.subtract, op1=ALU.mult,
        )

        # small per-layer params as per-partition column banks
        def col_bank(tag, dram, inner, pat):
            t = consts.tile([P, L, inner] if pat == "l" else [P, inner], F32, tag=tag)
            if pat == "l":
                nc.sync.dma_start(out=t[:], in_=dram[:].rearrange("l (k p) -> p l k", p=P))
            else:
                nc.sync.dma_start(out=t[:], in_=dram[:].rearrange("(k p) -> p k", p=P))
            return t

        ln1w = col_bank("ln1w", ln1w_d, KD, "l")
        ln1b = col_bank("ln1b", ln1b_d, KD, "l")
        ln2w = col_bank("ln2w", ln2w_d, KD, "l")
        ln2b = col_bank("ln2b", ln2b_d, KD, "l")
        bo_sb = col_bank("bo", bo_d, KD, "l")
        b2_sb = col_bank("b2", b2_d, KD, "l")
        b1_sb = col_bank("b1", b1_d, KF, "l")
        lnfw = col_bank("lnfw", lnfw_d, KD, "f")
        lnfb = col_bank("lnfb", lnfb_d, KD, "f")
        cb1_sb = col_bank("cb1", cb1_d, KF, "f")
        cb2_sb = consts.tile([1, NCLS], F32, tag="cb2")
        nc.sync.dma_start(out=cb2_sb[:], in_=cb2_d[None, :])

        # residual stream + post-LN activations, persistent
        h_pool = ctx.enter_context(tc.tile_pool(name="h", bufs=1))
        h_B = [h_pool.tile([P, S], F32, tag=f"h{k}", name=f"h{k}") for k in range(KD)]
        xn_pool = ctx.enter_context(tc.tile_pool(name="xn", bufs=1))
        xn = [xn_pool.tile([P, S], BF16, tag=f"xn{k}", name=f"xn{k}") for k in range(KD)]

        # ---------------- embedding ----------------
        with tc.tile_pool(name="emb", bufs=3) as emb, tc.tile_pool(
            name="emb_ps", bufs=2, space="PSUM"
        ) as emb_ps:
            for t in range(NT):
                gat = emb.tile([P, D], F32, tag="gat")
                nc.gpsimd.indirect_dma_start(
                    out=gat[:],
                    out_offset=None,
                    in_=temb_d[:],
                    in_offset=bass.IndirectOffsetOnAxis(ap=ids_sb[:, t : t + 1], axis=0),
                )
                pos = emb.tile([P, D], F32, tag="pos")
                nc.sync.dma_start(out=pos[:], in_=pemb_d[t * P : (t + 1) * P, :])
                ha = emb.tile([P, D], F32, tag="ha")
                nc.vector.tensor_add(out=ha[:], in0=gat[:], in1=pos[:])
                for k in range(KD):
                    pst = emb_ps.tile([P, P], F32, tag="pst")
                    nc.tensor.transpose(
                        out=pst[:], in_=ha[:, k * P : (k + 1) * P], identity=ident[:]
                    )
                    nc.vector.tensor_copy(
                        out=h_B[k][:, t * P : (t + 1) * P], in_=pst[:]
                    )

        # ---------------- layernorm helper ----------------
        def layernorm(li, w_bank, b_bank, dst):
            """dst[k] (bf16) = LN(h_B) * w + b; feature axis = partitions."""
            with tc.tile_pool(name=f"ln{li}", bufs=2) as lnp, tc.tile_pool(
                name=f"ln{li}s", bufs=2
            ) as lns, tc.tile_pool(name=f"ln{li}_ps", bufs=2, space="PSUM") as psp:
                for qb in range(NQ):
                    qs = slice(qb * QBS, (qb + 1) * QBS)
                    ps_sum = psp.tile([P, QBS], F32, tag="pssum")
                    ps_sq = psp.tile([P, QBS], F32, tag="pssq")
                    for k in range(KD):
                        hb = lnp.tile([P, QBS], BF16, tag="hb")
                        nc.vector.tensor_copy(out=hb[:], in_=h_B[k][:, qs])
                        nc.tensor.matmul(
                            ps_sum[:], ones_bf[:], hb[:],
                            start=(k == 0), stop=(k == KD - 1),
                        )
                        sq = lnp.tile([P, QBS], BF16, tag="sq")
                        nc.vector.tensor_mul(out=sq[:], in0=hb[:], in1=hb[:])
                        nc.tensor.matmul(
                            ps_sq[:], ones_bf[:], sq[:],
                            start=(k == 0), stop=(k == KD - 1),
                        )
                    mean = lns.tile([P, QBS], F32, tag="mean")
                    nc.vector.tensor_scalar_mul(out=mean[:], in0=ps_sum[:], scalar1=1.0 / D)
                    msq = lnp.tile([P, QBS], F32, tag="msq")
                    nc.vector.tensor_scalar_mul(out=msq[:], in0=ps_sq[:], scalar1=1.0 / D)
                    var = lnp.tile([P, QBS], F32, tag="var")
                    nc.vector.tensor_mul(out=var[:], in0=mean[:], in1=mean[:])
                    nc.vector.tensor_sub(out=var[:], in0=msq[:], in1=var[:])
                    std = lnp.tile([P, QBS], F32, tag="std")
                    nc.scalar.activation(out=std[:], in_=var[:], func=AF.Sqrt, bias=eps_col[:])
                    rstd = lns.tile([P, QBS], F32, tag="rstd")
                    nc.vector.reciprocal(out=rstd[:], in_=std[:])
                    for k in range(KD):
                        tmp = lnp.tile([P, QBS], F32, tag="tmp")
                        nc.vector.tensor_sub(out=tmp[:], in0=h_B[k][:, qs], in1=mean[:])
                        nc.vector.tensor_mul(out=dst[k][:, qs], in0=tmp[:], in1=rstd[:])
                        nc.vector.tensor_scalar(
                            out=dst[k][:, qs], in0=dst[k][:, qs],
                            scalar1=w_bank[:, k : k + 1], scalar2=b_bank[:, k : k + 1],
                            op0=ALU.mult, op1=ALU.add,
                        )

        # ---------------- layers ----------------
        for l in range(L):
            layernorm(f"1_{l}", ln1w[:, l, :], ln1b[:, l, :], xn)

            with ExitStack() as lctx:
                qkvw = lctx.enter_context(tc.tile_pool(name=f"qkvw{l}", bufs=1))
                wst = lctx.enter_context(tc.tile_pool(name=f"wst{l}", bufs=3))
                qkp = lctx.enter_context(tc.tile_pool(name=f"qk{l}", bufs=1))
                vp = lctx.enter_context(tc.tile_pool(name=f"v{l}", bufs=1))
                attop = lctx.enter_context(tc.tile_pool(name=f"atto{l}", bufs=1))

                # --- qkv projections ---
                with tc.tile_pool(name=f"qkv_ps{l}", bufs=3, space="PSUM") as qkps:
                    w_b = {}
                    for name, dram in (("q", wq_d), ("k", wk_d), ("v", wv_d)):
                        wb = qkvw.tile([P, KD, H * HS], BF16, tag=f"w{name}b")
                        w_b[name] = wb
                        for k in range(KD):
                            st = wst.tile([P, H * HS], F32, tag="wst")
                            nc.sync.dma_start(
                                out=st[:].rearrange("p (h e) -> p h e", e=HS),
                                in_=dram[l][:, k * P : (k + 1) * P, :].rearrange(
                                    "h p e -> p h e"
                                ),
                            )
                            nc.scalar.copy(out=wb[:, k, :], in_=st[:])

                    q_pair = [qkp.tile([P, S], BF16, tag=f"q{i}", name=f"q{i}") for i in range(HP)]
                    k_pair = [qkp.tile([P, S], BF16, tag=f"k{i}", name=f"k{i}") for i in range(HP)]
                    for name, dest in (("q", q_pair), ("k", k_pair)):
                        wb = w_b[name]
                        for i in range(HP):
                            for qb in range(NQ):
                                qs = slice(qb * QBS, (qb + 1) * QBS)
                                ps = qkps.tile([P, QBS], F32, tag="psqk")
                                for k in range(KD):
                                    st0, sp0 = (k == 0), (k == KD - 1)
                                    nc.tensor.matmul(
                                        ps[0:HS, :],
                                        wb[:, k, (2 * i) * HS : (2 * i + 1) * HS],
                                        xn[k][:, qs],
                                        start=st0, stop=sp0,
                                        tile_position=(0, 0),
                                        skip_group_check=True,
                                    )
                                    nc.tensor.matmul(
                                        ps[HS : 2 * HS, :],
                                        wb[:, k, (2 * i + 1) * HS : (2 * i + 2) * HS],
                                        xn[k][:, qs],
                                        start=st0, stop=sp0,
                                        tile_position=(0, HS),
                                        skip_group_check=True,
                                    )
                                nc.scalar.copy(out=dest[i][:, qs], in_=ps[:])

                    v_all = [vp.tile([P, H, HS + 1], BF16, tag=f"v{t}", name=f"v{t}") for t in range(NT)]
                    wvb = w_b["v"]
                    nsplits = []
                    off = 0
                    while off < H * HS:
                        nsz = min(512, H * HS - off)
                        nsplits.append((off, nsz))
                        off += nsz
                    for t in range(NT):
                        nc.vector.memset(v_all[t][:, :, HS : HS + 1], 1.0)
                        for noff, nsz in nsplits:
                            ps = qkps.tile([P, 512], F32, tag="psv")
                            for k in range(KD):
                                nc.tensor.matmul(
                                    ps[:, :nsz],
                                    xn[k][:, t * P : (t + 1) * P],
                                    wvb[:, k, noff : noff + nsz],
                                    start=(k == 0), stop=(k == KD - 1),
                                )
                            h0 = noff // HS
                            nh = nsz // HS
                            nc.scalar.copy(
                                out=v_all[t][:, h0 : h0 + nh, 0:HS],
                                in_=ps[:, :nsz].rearrange("p (h e) -> p h e", e=HS),
                            )

                # --- attention ---
                atto = [attop.tile([P, S], BF16, tag=f"ao{i}", name=f"ao{i}") for i in range(HP)]
                with tc.tile_pool(name=f"att{l}", bufs=2 * NT) as attp, tc.tile_pool(
                    name=f"attsm{l}", bufs=3
                ) as attsm, tc.tile_pool(
                    name=f"att_pss{l}", bufs=3, space="PSUM"
                ) as attps, tc.tile_pool(
                    name=f"att_psb{l}", bufs=1, space="PSUM"
                ) as attpsb, tc.tile_pool(
                    name=f"att_psav{l}", bufs=2, space="PSUM"
                ) as attps2:
                    for hd in range(H):
                        pi, r0 = hd // 2, (hd % 2) * HS
                        for qb in range(NQ):
                            qs = slice(qb * QBS, (qb + 1) * QBS)
                            att_tiles = []
                            for kt in range(NT):
                                ps_s = attps.tile([P, QBS], F32, tag="pss")
                                nc.tensor.matmul(
                                    ps_s[:],
                                    k_pair[pi][r0 : r0 + HS, kt * P : (kt + 1) * P],
                                    q_pair[pi][r0 : r0 + HS, qs],
                                    start=True, stop=True,
                                )
                                at = attp.tile([P, QBS], BF16, tag="attT")
                                nc.scalar.activation(
                                    out=at[:], in_=ps_s[:], func=AF.Exp,
                                    bias=mbias[:, kt : kt + 1], scale=SCALE,
                                )
                                att_tiles.append(at)
                            ps_av = attps2.tile([P, QBS], F32, tag="psav")
                            for kt in range(NT):
                                nc.tensor.matmul(
                                    ps_av[0 : HS + 1, :],
                                    v_all[kt][:, hd, :],
                                    att_tiles[kt][:],
                                    start=(kt == 0), stop=(kt == NT - 1),
                                )
                            rden = attsm.tile([1, QBS], BF16, tag="rden")
                            with nc.allow_low_precision(reason="softmax denom bf16"):
                                nc.vector.reciprocal(out=rden[:], in_=ps_av[HS : HS + 1, :])
                            ps_bc = attpsb.tile([P, QBS], F32, tag="psbc")
                            nc.tensor.matmul(
                                ps_bc[0:HS, :], ones_bf[0:1, 0:HS], rden[:],
                                start=True, stop=True,
                            )
                            rb = attsm.tile([HS, QBS], BF16, tag="rb")
                            nc.scalar.copy(out=rb[:], in_=ps_bc[0:HS, :])
                            nc.vector.tensor_mul(
                                out=atto[pi][r0 : r0 + HS, qs],
                                in0=ps_av[0:HS, :], in1=rb[:],
                            )

                # --- output projection + residual ---
                with tc.tile_pool(name=f"wo{l}", bufs=1) as wop, tc.tile_pool(
                    name=f"wo_ps{l}", bufs=3, space="PSUM"
                ) as wops:
                    wob = wop.tile([P, KD, D], BF16, tag="wob")
                    for k in range(KD):
                        st = wst.tile([P, D], F32, tag="wst")
                        nc.sync.dma_start(
                            out=st[:], in_=wo_d[l][k * P : (k + 1) * P, :]
                        )
                        nc.scalar.copy(out=wob[:, k, :], in_=st[:])
                    for do in range(KD):
                        for qb in range(NQ):
                            qs = slice(qb * QBS, (qb + 1) * QBS)
                            ps = wops.tile([P, QBS], F32, tag="pswo")
                            for di in range(KD):
                                nc.tensor.matmul(
                                    ps[:],
                                    wob[:, di, do * P : (do + 1) * P],
                                    atto[di][:, qs],
                                    start=(di == 0), stop=(di == KD - 1),
                                )
                            tt = wst.tile([P, QBS], F32, tag="two")
                            nc.scalar.activation(
                                out=tt[:], in_=ps[:], func=AF.Identity,
                                bias=bo_sb[:, l, do : do + 1],
                            )
                            nc.vector.tensor_add(
                                out=h_B[do][:, qs], in0=h_B[do][:, qs], in1=tt[:]
                            )

            layernorm(f"2_{l}", ln2w[:, l, :], ln2b[:, l, :], xn)

            # --- MLP ---
            with tc.tile_pool(name=f"w1p{l}", bufs=1) as w1p, tc.tile_pool(
                name=f"w1st{l}", bufs=3
            ) as w1st, tc.tile_pool(name=f"ffp{l}", bufs=1) as ffp, tc.tile_pool(
                name=f"w2p{l}", bufs=4
            ) as w2p, tc.tile_pool(
                name=f"mlp_ps{l}", bufs=2, space="PSUM"
            ) as mlps, tc.tile_pool(
                name=f"mlp_ps2{l}", bufs=1, space="PSUM"
            ) as mlps2:
                w1b = [w1p.tile([P, FF], BF16, tag=f"w1b{k}", name=f"w1b{k}") for k in range(KD)]
                for k in range(KD):
                    st = w1st.tile([P, FF], F32, tag="w1st")
                    nc.sync.dma_start(out=st[:], in_=w1_d[l][k * P : (k + 1) * P, :])
                    nc.gpsimd.tensor_copy(out=w1b[k][:], in_=st[:])
                ff = [ffp.tile([P, QBS], BF16, tag=f"ff{m}", name=f"ff{m}") for m in range(KF)]
                for qb in range(NQ):
                    qs = slice(qb * QBS, (qb + 1) * QBS)
                    for m in range(KF):
                        ps = mlps.tile([P, QBS], F32, tag="psw1")
                        for k in range(KD):
                            nc.tensor.matmul(
                                ps[:],
                                w1b[k][:, m * P : (m + 1) * P],
                                xn[k][:, qs],
                                start=(k == 0), stop=(k == KD - 1),
                            )
                        nc.scalar.activation(
                            out=ff[m][:], in_=ps[:], func=AF.Relu,
                            bias=b1_sb[:, l, m : m + 1],
                        )
                    # W2: k2-outer accumulation into KD open psums
                    ps_o = [mlps2.tile([P, QBS], F32, tag=f"psw2_{do}", name=f"psw2_{do}") for do in range(KD)]
                    for k2 in range(KF):
                        w2b = w2p.tile([P, D], BF16, tag="w2b")
                        st = w1st.tile([P, D], F32, tag="w2st")
                        nc.sync.dma_start(
                            out=st[:], in_=w2_d[l][k2 * P : (k2 + 1) * P, :]
                        )
                        nc.gpsimd.tensor_copy(out=w2b[:], in_=st[:])
                        for do in range(KD):
                            nc.tensor.matmul(
                                ps_o[do][:],
                                w2b[:, do * P : (do + 1) * P],
                                ff[k2][:],
                                start=(k2 == 0), stop=(k2 == KF - 1),
                            )
                    for do in range(KD):
                        tt = w1st.tile([P, QBS], F32, tag="tw2")
                        nc.scalar.activation(
                            out=tt[:], in_=ps_o[do][:], func=AF.Identity,
                            bias=b2_sb[:, l, do : do + 1],
                        )
                        nc.vector.tensor_add(
                            out=h_B[do][:, qs], in0=h_B[do][:, qs], in1=tt[:]
                        )

        # ---------------- final LN (last token) + classifier ----------------
        with tc.tile_pool(name="fin", bufs=1) as fin, tc.tile_pool(
            name="finst", bufs=3
        ) as finst, tc.tile_pool(name="fin_ps", bufs=1, space="PSUM") as finps:
            hcb = fin.tile([P, KD], BF16, tag="hcb")
            sqc = fin.tile([P, KD], BF16, tag="sqc")
            for k in range(KD):
                nc.vector.tensor_copy(out=hcb[:, k : k + 1], in_=h_B[k][:, S - 1 : S])
                nc.vector.tensor_mul(
                    out=sqc[:, k : k + 1],
                    in0=h_B[k][:, S - 1 : S], in1=h_B[k][:, S - 1 : S],
                )
            ps_sum = finps.tile([P, 1], F32, tag="fsum")
            ps_sq = finps.tile([P, 1], F32, tag="fsq")
            for k in range(KD):
                nc.tensor.matmul(
                    ps_sum[:], ones_bf[:], hcb[:, k : k + 1],
                    start=(k == 0), stop=(k == KD - 1),
                )
                nc.tensor.matmul(
                    ps_sq[:], ones_bf[:], sqc[:, k : k + 1],
                    start=(k == 0), stop=(k == KD - 1),
                )
            mean = fin.tile([P, 1], F32, tag="fmean")
            nc.vector.tensor_scalar_mul(out=mean[:], in0=ps_sum[:], scalar1=1.0 / D)
            msq = fin.tile([P, 1], F32, tag="fmsq")
            nc.vector.tensor_scalar_mul(out=msq[:], in0=ps_sq[:], scalar1=1.0 / D)
            var = fin.tile([P, 1], F32, tag="fvar")
            nc.vector.tensor_mul(out=var[:], in0=mean[:], in1=mean[:])
            nc.vector.tensor_sub(out=var[:], in0=msq[:], in1=var[:])
            std = fin.tile([P, 1], F32, tag="fstd")
            nc.scalar.activation(out=std[:], in_=var[:], func=AF.Sqrt, bias=eps_col[:])
            rstd = fin.tile([P, 1], F32, tag="frstd")
            nc.vector.reciprocal(out=rstd[:], in_=std[:])
            xnl = fin.tile([P, KD], BF16, tag="xnl")
            for k in range(KD):
                tmp = finst.tile([P, 1], F32, tag="ftmp")
                nc.vector.tensor_sub(out=tmp[:], in0=h_B[k][:, S - 1 : S], in1=mean[:])
                nc.vector.tensor_mul(out=xnl[:, k : k + 1], in0=tmp[:], in1=rstd[:])
                nc.vector.tensor_scalar(
                    out=xnl[:, k : k + 1], in0=xnl[:, k : k + 1],
                    scalar1=lnfw[:, k : k + 1], scalar2=lnfb[:, k : k + 1],
                    op0=ALU.mult, op1=ALU.add,
                )
            # classifier mlp
            c1b = [fin.tile([P, FF], BF16, tag=f"c1b{k}", name=f"c1b{k}") for k in range(KD)]
            for k in range(KD):
                st = finst.tile([P, FF], F32, tag="cst")
                nc.sync.dma_start(out=st[:], in_=cw1_d[k * P : (k + 1) * P, :])
                nc.gpsimd.tensor_copy(out=c1b[k][:], in_=st[:])
            hidT = fin.tile([P, KF], BF16, tag="hidT")
            for m in range(KF):
                ps_h = finps.tile([P, 1], F32, tag="fh")
                for k in range(KD):
                    nc.tensor.matmul(
                        ps_h[:], c1b[k][:, m * P : (m + 1) * P], xnl[:, k : k + 1],
                        start=(k == 0), stop=(k == KD - 1),
                    )
                nc.scalar.activation(
                    out=hidT[:, m : m + 1], in_=ps_h[:], func=AF.Relu,
                    bias=cb1_sb[:, m : m + 1],
                )
            c2st = fin.tile([P, KF, NCLS], F32, tag="c2st")
            nc.sync.dma_start(
                out=c2st[:], in_=cw2_d[:].rearrange("(k p) c -> p k c", p=P)
            )
            c2b = fin.tile([P, KF, NCLS], BF16, tag="c2b")
            nc.vector.tensor_copy(out=c2b[:], in_=c2st[:])
            ps_l = finps.tile([1, NCLS], F32, tag="flog")
            for k2 in range(KF):
                nc.tensor.matmul(
                    ps_l[:], hidT[:, k2 : k2 + 1], c2b[:, k2, :],
                    start=(k2 == 0), stop=(k2 == KF - 1),
                )
            out_sb = fin.tile([1, NCLS], F32, tag="outsb")
            nc.vector.tensor_add(out=out_sb[:], in0=ps_l[:], in1=cb2_sb[:])
            nc.sync.dma_start(out=out_d[:], in_=out_sb[:])

    nc.finalize()
    return nc


_NC_CACHE = {}


def _get_nc(**kw):
    key = tuple(sorted(kw.items()))
    if key not in _NC_CACHE:
        _NC_CACHE[key] = build_nc(**kw)
    return _NC_CACHE[key]


def kernel(**inputs):
    """Full-model forward: takes the unsharded inputs from setup_inputs(),
    runs data-parallel across 8 NeuronCores, returns [B, NCLS] f32 logits."""
    x = np.ascontiguousarray(np.asarray(inputs["x"]), dtype=np.int32)
    mask = np.ascontiguousarray(np.asarray(inputs["attention_mask"]), dtype=np.int32)
    B = x.shape[0]
    f32 = lambda name: np.ascontiguousarray(np.asarray(inputs[name]), dtype=np.float32)
    weights = {
        name: f32(name)
        for name in (
            "tok_emb", "pos_emb", "Wq", "Wk", "Wv", "Wo", "bo",
            "ln1_w", "ln1_b", "ln2_w", "ln2_b", "W1", "b1", "W2", "b2",
            "lnf_w", "lnf_b", "cW1", "cb1", "cW2", "cb2",
        )
    }
    nc = _get_nc()
    in_maps = []
    for c in range(B):
        m = {"ids": x[c], "mask": mask[c]}
        m.update(weights)
        in_maps.append(m)
    res = run_bass_kernel_spmd(nc, in_maps, list(range(B)))
    return np.concatenate([res.results[c]["out"] for c in range(B)], axis=0)


# revision 12
# speedup vs baseline: 1.2245x; 1.2245x over previous
"""Trainium2 Bass kernel for a 4-layer GPT classifier (CMGPTClassifier).

Strategy: data-parallel over batch — each of the 8 NeuronCores runs the full
model on one sequence. All activations stay resident in SBUF in a
"layout B" = [feature-on-partitions, tokens-in-free] layout; weights stream
from HBM as casting-DMAs (f32 in DRAM -> bf16 in SBUF, software DGE);
matmuls run in bf16 with f32 PSUM accumulation.

Model (per core): S=1024 tokens, D=768, H=12 heads (HS=64), FF=3072, L=4
layers, 16 classes. h = tok_emb[x] + pos_emb; per layer:
  xn  = LN1(h);  q,k,v per head;  att = softmax(q k^T / sqrt(D)) v
  h  += concat(att) @ Wo + bo
  xn2 = LN2(h);  h += relu(xn2 @ W1 + b1) @ W2 + b2
logits = relu(LNf(h)[last] @ cW1 + cb1) @ cW2 + cb2

Implementation notes:
 - LN over the partition (feature) axis uses a ones[128,128] stationary
   matmul to produce per-token sums broadcast across partitions, so all the
   stat math runs as full-width [128, 512] vector ops.
 - Attention works in the transposed layout: scores_T[key, query] tiles, two
   heads of a pair issued back-to-back into opposite PE row-groups so they
   run concurrently; exp runs on ScalarE straight out of PSUM with the
   attention scale and key-mask bias folded in. The softmax denominator
   comes from a ones-column appended to V (lhsT [128, 65]); the att@V stage
   runs one iteration behind the scores/exp stage so the PE never stalls on
   ScalarE. Normalization multiplies by a PE-broadcast fast-reciprocal row.
"""

from contextlib import ExitStack

import numpy as np

import concourse.bacc as bacc
import concourse.bass as bass
import concourse.mybir as mybir
import concourse.tile as tile
from concourse.bass_utils import run_bass_kernel_spmd
from concourse.masks import make_identity

F32 = mybir.dt.float32
BF16 = mybir.dt.bfloat16
I32 = mybir.dt.int32
AF = mybir.ActivationFunctionType
ALU = mybir.AluOpType

P = 128


def build_nc(S=1024, L=4, H=12, D=768, FF=3072, V=32000, NCLS=16, cast_dma=True):
    HS = D // H
    KD = D // P          # 6 feature tiles
    KF = FF // P         # 24 ff tiles
    NT = S // P          # token tiles
    QBS = min(512, S)    # token block for matmul free dim
    NQ = S // QBS
    HP = H // 2          # head pairs
    SCALE = float(D) ** -0.5

    nc = bacc.Bacc("TRN2", target_bir_lowering=False)

    ids_d = nc.dram_tensor("ids", [S], I32, kind="ExternalInput")
    mask_d = nc.dram_tensor("mask", [S], I32, kind="ExternalInput")
    temb_d = nc.dram_tensor("tok_emb", [V, D], F32, kind="ExternalInput")
    pemb_d = nc.dram_tensor("pos_emb", [S, D], F32, kind="ExternalInput")
    wq_d = nc.dram_tensor("Wq", [L, H, D, HS], F32, kind="ExternalInput")
    wk_d = nc.dram_tensor("Wk", [L, H, D, HS], F32, kind="ExternalInput")
    wv_d = nc.dram_tensor("Wv", [L, H, D, HS], F32, kind="ExternalInput")
    wo_d = nc.dram_tensor("Wo", [L, D, D], F32, kind="ExternalInput")
    bo_d = nc.dram_tensor("bo", [L, D], F32, kind="ExternalInput")
    ln1w_d = nc.dram_tensor("ln1_w", [L, D], F32, kind="ExternalInput")
    ln1b_d = nc.dram_tensor("ln1_b", [L, D], F32, kind="ExternalInput")
    ln2w_d = nc.dram_tensor("ln2_w", [L, D], F32, kind="ExternalInput")
    ln2b_d = nc.dram_tensor("ln2_b", [L, D], F32, kind="ExternalInput")
    w1_d = nc.dram_tensor("W1", [L, D, FF], F32, kind="ExternalInput")
    b1_d = nc.dram_tensor("b1", [L, FF], F32, kind="ExternalInput")
    w2_d = nc.dram_tensor("W2", [L, FF, D], F32, kind="ExternalInput")
    b2_d = nc.dram_tensor("b2", [L, D], F32, kind="ExternalInput")
    lnfw_d = nc.dram_tensor("lnf_w", [D], F32, kind="ExternalInput")
    lnfb_d = nc.dram_tensor("lnf_b", [D], F32, kind="ExternalInput")
    cw1_d = nc.dram_tensor("cW1", [D, FF], F32, kind="ExternalInput")
    cb1_d = nc.dram_tensor("cb1", [FF], F32, kind="ExternalInput")
    cw2_d = nc.dram_tensor("cW2", [FF, NCLS], F32, kind="ExternalInput")
    cb2_d = nc.dram_tensor("cb2", [NCLS], F32, kind="ExternalInput")
    out_d = nc.dram_tensor("out", [1, NCLS], F32, kind="ExternalOutput")

    with tile.TileContext(nc) as tc, ExitStack() as ctx:
        consts = ctx.enter_context(tc.tile_pool(name="consts", bufs=1))
        ones_bf = consts.tile([P, P], BF16, tag="ones")
        nc.vector.memset(ones_bf[:], 1.0)
        ones_f = consts.tile([1, HS], F32, tag="onesf")
        nc.vector.memset(ones_f[:], 1.0)
        ident = consts.tile([P, P], F32, tag="ident")
        make_identity(nc, ident[:])
        eps_col = consts.tile([P, 1], F32, tag="eps")
        nc.vector.memset(eps_col[:], 1e-5)

        wstage_pool = (
            ctx.enter_context(tc.tile_pool(name="wstage", bufs=2))
            if not cast_dma else None
        )

        def load_cast(out_ap, in_ap, eng=None):
            """f32 DRAM -> bf16 SBUF; casting DMA or staged DMA + engine cast."""
            if cast_dma:
                nc.gpsimd.dma_start(out=out_ap, in_=in_ap)
                return
            np_, nf = out_ap.shape[0], out_ap.size() // out_ap.shape[0]
            st = wstage_pool.tile([np_, nf], F32, tag="wstage", name="wstage")
            shp = list(out_ap.shape)
            stv = st[:] if len(shp) == 2 else st[:].rearrange(
                "p (a b) -> p a b", a=shp[1], b=shp[2])
            nc.sync.dma_start(out=stv, in_=in_ap)
            if eng is None:
                nc.scalar.copy(out=out_ap, in_=stv)
            else:
                eng.tensor_copy(out=out_ap, in_=stv)

        ids_sb = consts.tile([P, NT], I32, tag="ids")
        nc.sync.dma_start(out=ids_sb[:], in_=ids_d[:].rearrange("(t p) -> p t", p=P))
        mask_sb = consts.tile([P, NT], I32, tag="mask")
        nc.sync.dma_start(out=mask_sb[:], in_=mask_d[:].rearrange("(t p) -> p t", p=P))
        maskf = consts.tile([P, NT], F32, tag="maskf")
        nc.vector.tensor_copy(out=maskf[:], in_=mask_sb[:])
        mbias = consts.tile([P, NT], F32, tag="mbias")
        nc.vector.tensor_scalar(
            out=mbias[:], in0=maskf[:], scalar1=1.0, scalar2=30.0,
            op0=ALU.subtract, op1=ALU.mult,
        )

        # small per-layer params as per-partition column banks
        def col_bank(tag, dram, inner, pat):
            t = consts.tile([P, L, inner] if pat == "l" else [P, inner], F32, tag=tag, name=tag)
            if pat == "l":
                nc.sync.dma_start(out=t[:], in_=dram[:].rearrange("l (k p) -> p l k", p=P))
            else:
                nc.sync.dma_start(out=t[:], in_=dram[:].rearrange("(k p) -> p k", p=P))
            return t

        ln1w = col_bank("ln1w", ln1w_d, KD, "l")
        ln1b = col_bank("ln1b", ln1b_d, KD, "l")
        ln2w = col_bank("ln2w", ln2w_d, KD, "l")
        ln2b = col_bank("ln2b", ln2b_d, KD, "l")
        bo_sb = col_bank("bo", bo_d, KD, "l")
        b2_sb = col_bank("b2", b2_d, KD, "l")
        b1_sb = col_bank("b1", b1_d, KF, "l")
        lnfw = col_bank("lnfw", lnfw_d, KD, "f")
        lnfb = col_bank("lnfb", lnfb_d, KD, "f")
        cb1_sb = col_bank("cb1", cb1_d, KF, "f")
        cb2_sb = consts.tile([1, NCLS], F32, tag="cb2")
        nc.sync.dma_start(out=cb2_sb[:], in_=cb2_d[None, :])

        # residual stream + post-LN activations, persistent
        h_pool = ctx.enter_context(tc.tile_pool(name="h", bufs=1))
        h_B = [h_pool.tile([P, S], F32, tag=f"h{k}", name=f"h{k}") for k in range(KD)]
        xn_pool = ctx.enter_context(tc.tile_pool(name="xn", bufs=1))
        xn = [xn_pool.tile([P, S], BF16, tag=f"xn{k}", name=f"xn{k}") for k in range(KD)]

        # ---------------- embedding ----------------
        with tc.tile_pool(name="emb", bufs=3) as emb, tc.tile_pool(
            name="emb_ps", bufs=2, space="PSUM"
        ) as emb_ps:
            for t in range(NT):
                gat = emb.tile([P, D], F32, tag="gat")
                nc.gpsimd.indirect_dma_start(
                    out=gat[:],
                    out_offset=None,
                    in_=temb_d[:],
                    in_offset=bass.IndirectOffsetOnAxis(ap=ids_sb[:, t : t + 1], axis=0),
                )
                pos = emb.tile([P, D], F32, tag="pos")
                nc.sync.dma_start(out=pos[:], in_=pemb_d[t * P : (t + 1) * P, :])
                ha = emb.tile([P, D], F32, tag="ha")
                nc.vector.tensor_add(out=ha[:], in0=gat[:], in1=pos[:])
                for k in range(KD):
                    pst = emb_ps.tile([P, P], F32, tag="pst")
                    nc.tensor.transpose(
                        out=pst[:], in_=ha[:, k * P : (k + 1) * P], identity=ident[:]
                    )
                    nc.vector.tensor_copy(
                        out=h_B[k][:, t * P : (t + 1) * P], in_=pst[:]
                    )

        # ---------------- layernorm helper ----------------
        def layernorm(li, w_bank, b_bank, dst):
            """dst[k] (bf16) = LN(h_B) * w + b; feature axis = partitions."""
            with tc.tile_pool(name=f"ln{li}", bufs=2) as lnp, tc.tile_pool(
                name=f"ln{li}s", bufs=2
            ) as lns, tc.tile_pool(name=f"ln{li}_ps", bufs=2, space="PSUM") as psp:
                for qb in range(NQ):
                    qs = slice(qb * QBS, (qb + 1) * QBS)
                    ps_sum = psp.tile([P, QBS], F32, tag="pssum")
                    ps_sq = psp.tile([P, QBS], F32, tag="pssq")
                    for k in range(KD):
                        hb = lnp.tile([P, QBS], BF16, tag="hb")
                        nc.vector.tensor_copy(out=hb[:], in_=h_B[k][:, qs])
                        nc.tensor.matmul(
                            ps_sum[:], ones_bf[:], hb[:],
                            start=(k == 0), stop=(k == KD - 1),
                        )
                        sq = lnp.tile([P, QBS], BF16, tag="sq")
                        nc.vector.tensor_mul(out=sq[:], in0=hb[:], in1=hb[:])
                        nc.tensor.matmul(
                            ps_sq[:], ones_bf[:], sq[:],
                            start=(k == 0), stop=(k == KD - 1),
                        )
                    mean = lns.tile([P, QBS], F32, tag="mean")
                    nc.vector.tensor_scalar_mul(out=mean[:], in0=ps_sum[:], scalar1=1.0 / D)
                    msq = lnp.tile([P, QBS], F32, tag="msq")
                    nc.vector.tensor_scalar_mul(out=msq[:], in0=ps_sq[:], scalar1=1.0 / D)
                    var = lnp.tile([P, QBS], F32, tag="var")
                    nc.vector.tensor_mul(out=var[:], in0=mean[:], in1=mean[:])
                    nc.vector.tensor_sub(out=var[:], in0=msq[:], in1=var[:])
                    std = lnp.tile([P, QBS], F32, tag="std")
                    nc.scalar.activation(out=std[:], in_=var[:], func=AF.Sqrt, bias=eps_col[:])
                    rstd = lns.tile([P, QBS], F32, tag="rstd")
                    nc.vector.reciprocal_approx_fast(out=rstd[:], in_=std[:])
                    for k in range(KD):
                        tmp = lnp.tile([P, QBS], F32, tag="tmp")
                        nc.vector.tensor_sub(out=tmp[:], in0=h_B[k][:, qs], in1=mean[:])
                        nc.vector.tensor_mul(out=dst[k][:, qs], in0=tmp[:], in1=rstd[:])
                        nc.vector.tensor_scalar(
                            out=dst[k][:, qs], in0=dst[k][:, qs],
                            scalar1=w_bank[:, k : k + 1], scalar2=b_bank[:, k : k + 1],
                            op0=ALU.mult, op1=ALU.add,
                        )

        # ---------------- layers ----------------
        for l in range(L):
            layernorm(f"1_{l}", ln1w[:, l, :], ln1b[:, l, :], xn)

            with ExitStack() as lctx:
                qkvw = lctx.enter_context(tc.tile_pool(name=f"qkvw{l}", bufs=1))
                qkp = lctx.enter_context(tc.tile_pool(name=f"qk{l}", bufs=1))
                vp = lctx.enter_context(tc.tile_pool(name=f"v{l}", bufs=1))
                attop = lctx.enter_context(tc.tile_pool(name=f"atto{l}", bufs=1))
                sst = lctx.enter_context(tc.tile_pool(name=f"sst{l}", bufs=3))

                # --- qkv projections (casting DMA: f32 DRAM -> bf16 SBUF) ---
                with tc.tile_pool(name=f"qkv_ps{l}", bufs=3, space="PSUM") as qkps:
                    w_b = {}
                    for name, dram in (("q", wq_d), ("k", wk_d), ("v", wv_d)):
                        wb = qkvw.tile([P, KD, H * HS], BF16, tag=f"w{name}b", name=f"w{name}b")
                        w_b[name] = wb
                        for k in range(KD):
                            load_cast(
                                wb[:, k, :].rearrange("p (h e) -> p h e", e=HS),
                                dram[l][:, k * P : (k + 1) * P, :].rearrange(
                                    "h p e -> p h e"
                                ),
                            )

                    q_pair = [qkp.tile([P, S], BF16, tag=f"q{i}", name=f"q{i}") for i in range(HP)]
                    k_pair = [qkp.tile([P, S], BF16, tag=f"k{i}", name=f"k{i}") for i in range(HP)]
                    for name, dest in (("q", q_pair), ("k", k_pair)):
                        wb = w_b[name]
                        for i in range(HP):
                            for qb in range(NQ):
                                qs = slice(qb * QBS, (qb + 1) * QBS)
                                ps = qkps.tile([P, QBS], F32, tag="psqk")
                                for k in range(KD):
                                    st0, sp0 = (k == 0), (k == KD - 1)
                                    nc.tensor.matmul(
                                        ps[0:HS, :],
                                        wb[:, k, (2 * i) * HS : (2 * i + 1) * HS],
                                        xn[k][:, qs],
                                        start=st0, stop=sp0,
                                        tile_position=(0, 0),
                                        skip_group_check=True,
                                    )
                                    nc.tensor.matmul(
                                        ps[HS : 2 * HS, :],
                                        wb[:, k, (2 * i + 1) * HS : (2 * i + 2) * HS],
                                        xn[k][:, qs],
                                        start=st0, stop=sp0,
                                        tile_position=(0, HS),
                                        skip_group_check=True,
                                    )
                                nc.vector.tensor_copy(out=dest[i][:, qs], in_=ps[:])

                    v_all = [vp.tile([P, H, HS + 1], BF16, tag=f"v{t}", name=f"v{t}") for t in range(NT)]
                    wvb = w_b["v"]
                    nsplits = []
                    off = 0
                    while off < H * HS:
                        nsz = min(512, H * HS - off)
                        nsplits.append((off, nsz))
                        off += nsz
                    for t in range(NT):
                        nc.vector.memset(v_all[t][:, :, HS : HS + 1], 1.0)
                        for noff, nsz in nsplits:
                            ps = qkps.tile([P, 512], F32, tag="psv")
                            for k in range(KD):
                                nc.tensor.matmul(
                                    ps[:, :nsz],
                                    xn[k][:, t * P : (t + 1) * P],
                                    wvb[:, k, noff : noff + nsz],
                                    start=(k == 0), stop=(k == KD - 1),
                                )
                            h0 = noff // HS
                            nh = nsz // HS
                            nc.scalar.copy(
                                out=v_all[t][:, h0 : h0 + nh, 0:HS],
                                in_=ps[:, :nsz].rearrange("p (h e) -> p h e", e=HS),
                            )

                # --- attention (scores/exp one iteration ahead of att@V) ---
                atto = [attop.tile([P, S], BF16, tag=f"ao{i}", name=f"ao{i}") for i in range(HP)]
                with tc.tile_pool(name=f"att{l}", bufs=3 * NT) as attp, tc.tile_pool(
                    name=f"attsm{l}", bufs=3
                ) as attsm, tc.tile_pool(
                    name=f"att_pss{l}", bufs=2, space="PSUM"
                ) as attps, tc.tile_pool(
                    name=f"att_psb{l}", bufs=2, space="PSUM"
                ) as attpsb, tc.tile_pool(
                    name=f"att_psav{l}", bufs=2, space="PSUM"
                ) as attps2:

                    def do_av(state):
                        pi, qb, ats = state
                        qs = slice(qb * QBS, (qb + 1) * QBS)
                        for ho in (0, 1):
                            hd, r0 = 2 * pi + ho, ho * HS
                            ps_av = attps2.tile([P, QBS], F32, tag="psav", name="psav")
                            for kt in range(NT):
                                nc.tensor.matmul(
                                    ps_av[0 : HS + 1, :],
                                    v_all[kt][:, hd, :],
                                    ats[ho][kt][:],
                                    start=(kt == 0), stop=(kt == NT - 1),
                                )
                            den = attsm.tile([1, QBS], F32, tag="den", name="den")
                            nc.scalar.copy(out=den[:], in_=ps_av[HS : HS + 1, :])
                            denr = attsm.tile([1, QBS], F32, tag="denr", name="denr")
                            nc.vector.reciprocal_approx_fast(out=denr[:], in_=den[:])
                            ps_bc = attpsb.tile([P, QBS], F32, tag="psbc", name="psbc")
                            nc.tensor.matmul(
                                ps_bc[0:HS, :], ones_f[:, :], denr[:],
                                start=True, stop=True,
                            )
                            avs = attsm.tile([HS, QBS], F32, tag="avs", name="avs")
                            nc.scalar.copy(out=avs[:], in_=ps_av[0:HS, :])
                            nc.vector.tensor_mul(
                                out=atto[pi][r0 : r0 + HS, qs],
                                in0=avs[:], in1=ps_bc[0:HS, :],
                            )

                    pend = None
                    for pi in range(HP):
                        for qb in range(NQ):
                            qs = slice(qb * QBS, (qb + 1) * QBS)
                            ats = ([], [])
                            for kt in range(NT):
                                for ho in (0, 1):
                                    r0 = ho * HS
                                    ps_s = attps.tile([P, QBS], F32, tag=f"pss{ho}", name=f"pss{ho}")
                                    nc.tensor.matmul(
                                        ps_s[:],
                                        k_pair[pi][r0 : r0 + HS, kt * P : (kt + 1) * P],
                                        q_pair[pi][r0 : r0 + HS, qs],
                                        start=True, stop=True,
                                    )
                                    at = attp.tile([P, QBS], BF16, tag="attT", name="attT")
                                    nc.scalar.activation(
                                        out=at[:], in_=ps_s[:], func=AF.Exp,
                                        bias=mbias[:, kt : kt + 1], scale=SCALE,
                                    )
                                    ats[ho].append(at)
                            if pend is not None:
                                do_av(pend)
                            pend = (pi, qb, ats)
                    do_av(pend)

                # --- output projection + residual ---
                with tc.tile_pool(name=f"wo{l}", bufs=1) as wop, tc.tile_pool(
                    name=f"wo_ps{l}", bufs=3, space="PSUM"
                ) as wops:
                    wob = wop.tile([P, KD, D], BF16, tag="wob")
                    load_cast(
                        wob[:],
                        wo_d[l][:, :].rearrange("(k p) m -> p k m", p=P),
                    )
                    for do in range(KD):
                        for qb in range(NQ):
                            qs = slice(qb * QBS, (qb + 1) * QBS)
                            ps = wops.tile([P, QBS], F32, tag="pswo")
                            for di in range(KD):
                                nc.tensor.matmul(
                                    ps[:],
                                    wob[:, di, do * P : (do + 1) * P],
                                    atto[di][:, qs],
                                    start=(di == 0), stop=(di == KD - 1),
                                )
                            tt = sst.tile([P, QBS], F32, tag="two")
                            nc.scalar.activation(
                                out=tt[:], in_=ps[:], func=AF.Identity,
                                bias=bo_sb[:, l, do : do + 1],
                            )
                            nc.vector.tensor_add(
                                out=h_B[do][:, qs], in0=h_B[do][:, qs], in1=tt[:]
                            )

            layernorm(f"2_{l}", ln2w[:, l, :], ln2b[:, l, :], xn)

            # --- MLP ---
            with tc.tile_pool(name=f"w1p{l}", bufs=1) as w1p, tc.tile_pool(
                name=f"mst{l}", bufs=3
            ) as mst, tc.tile_pool(name=f"ffp{l}", bufs=1) as ffp, tc.tile_pool(
                name=f"w2p{l}", bufs=4
            ) as w2p, tc.tile_pool(
                name=f"mlp_ps{l}", bufs=2, space="PSUM"
            ) as mlps, tc.tile_pool(
                name=f"mlp_ps2{l}", bufs=1, space="PSUM"
            ) as mlps2:
                w1b = [w1p.tile([P, FF], BF16, tag=f"w1b{k}", name=f"w1b{k}") for k in range(KD)]
                for k in range(KD):
                    load_cast(w1b[k][:], w1_d[l][k * P : (k + 1) * P, :], eng=nc.gpsimd)
                ff = [ffp.tile([P, QBS], BF16, tag=f"ff{m}", name=f"ff{m}") for m in range(KF)]
                for qb in range(NQ):
                    qs = slice(qb * QBS, (qb + 1) * QBS)
                    for m in range(KF):
                        ps = mlps.tile([P, QBS], F32, tag="psw1")
                        for k in range(KD):
                            nc.tensor.matmul(
                                ps[:],
                                w1b[k][:, m * P : (m + 1) * P],
                                xn[k][:, qs],
                                start=(k == 0), stop=(k == KD - 1),
                            )
                        # relu(x + b1) on DVE: add bias column, clamp at 0
                        nc.vector.tensor_scalar(
                            out=ff[m][:], in0=ps[:],
                            scalar1=b1_sb[:, l, m : m + 1], scalar2=0.0,
                            op0=ALU.add, op1=ALU.max,
                        )
                    # W2: k2-outer accumulation into KD open psums
                    ps_o = [mlps2.tile([P, QBS], F32, tag=f"psw2_{do}", name=f"psw2_{do}") for do in range(KD)]
                    for k2 in range(KF):
                        w2b = w2p.tile([P, D], BF16, tag="w2b")
                        load_cast(w2b[:], w2_d[l][k2 * P : (k2 + 1) * P, :], eng=nc.gpsimd)
                        for do in range(KD):
                            nc.tensor.matmul(
                                ps_o[do][:],
                                w2b[:, do * P : (do + 1) * P],
                                ff[k2][:],
                                start=(k2 == 0), stop=(k2 == KF - 1),
                            )
                    for do in range(KD):
                        tt = mst.tile([P, QBS], F32, tag="tw2")
                        nc.scalar.activation(
                            out=tt[:], in_=ps_o[do][:], func=AF.Identity,
                            bias=b2_sb[:, l, do : do + 1],
                        )
                        nc.vector.tensor_add(
                            out=h_B[do][:, qs], in0=h_B[do][:, qs], in1=tt[:]
                        )

        # ---------------- final LN (last token) + classifier ----------------
        with tc.tile_pool(name="fin", bufs=1) as fin, tc.tile_pool(
            name="finst", bufs=3
        ) as finst, tc.tile_pool(name="fin_ps", bufs=1, space="PSUM") as finps:
            hcb = fin.tile([P, KD], BF16, tag="hcb")
            sqc = fin.tile([P, KD], BF16, tag="sqc")
            for k in range(KD):
                nc.vector.tensor_copy(out=hcb[:, k : k + 1], in_=h_B[k][:, S - 1 : S])
                nc.vector.tensor_mul(
                    out=sqc[:, k : k + 1],
                    in0=h_B[k][:, S - 1 : S], in1=h_B[k][:, S - 1 : S],
                )
            ps_sum = finps.tile([P, 1], F32, tag="fsum")
            ps_sq = finps.tile([P, 1], F32, tag="fsq")
            for k in range(KD):
                nc.tensor.matmul(
                    ps_sum[:], ones_bf[:], hcb[:, k : k + 1],
                    start=(k == 0), stop=(k == KD - 1),
                )
                nc.tensor.matmul(
                    ps_sq[:], ones_bf[:], sqc[:, k : k + 1],
                    start=(k == 0), stop=(k == KD - 1),
                )
            mean = fin.tile([P, 1], F32, tag="fmean")
            nc.vector.tensor_scalar_mul(out=mean[:], in0=ps_sum[:], scalar1=1.0 / D)
            msq = fin.tile([P, 1], F32, tag="fmsq")
            nc.vector.tensor_scalar_mul(out=msq[:], in0=ps_sq[:], scalar1=1.0 / D)
            var = fin.tile([P, 1], F32, tag="fvar")
            nc.vector.tensor_mul(out=var[:], in0=mean[:], in1=mean[:])
            nc.vector.tensor_sub(out=var[:], in0=msq[:], in1=var[:])
            std = fin.tile([P, 1], F32, tag="fstd")
            nc.scalar.activation(out=std[:], in_=var[:], func=AF.Sqrt, bias=eps_col[:])
            rstd = fin.tile([P, 1], F32, tag="frstd")
            nc.vector.reciprocal_approx_fast(out=rstd[:], in_=std[:])
            xnl = fin.tile([P, KD], BF16, tag="xnl")
            for k in range(KD):
                tmp = finst.tile([P, 1], F32, tag="ftmp")
                nc.vector.tensor_sub(out=tmp[:], in0=h_B[k][:, S - 1 : S], in1=mean[:])
                nc.vector.tensor_mul(out=xnl[:, k : k + 1], in0=tmp[:], in1=rstd[:])
                nc.vector.tensor_scalar(
                    out=xnl[:, k : k + 1], in0=xnl[:, k : k + 1],
                    scalar1=lnfw[:, k : k + 1], scalar2=lnfb[:, k : k + 1],
                    op0=ALU.mult, op1=ALU.add,
                )
            # classifier mlp
            c1b = [fin.tile([P, FF], BF16, tag=f"c1b{k}", name=f"c1b{k}") for k in range(KD)]
            for k in range(KD):
                load_cast(c1b[k][:], cw1_d[k * P : (k + 1) * P, :], eng=nc.gpsimd)
            hidT = fin.tile([P, KF], BF16, tag="hidT")
            for m in range(KF):
                ps_h = finps.tile([P, 1], F32, tag="fh")
                for k in range(KD):
                    nc.tensor.matmul(
                        ps_h[:], c1b[k][:, m * P : (m + 1) * P], xnl[:, k : k + 1],
                        start=(k == 0), stop=(k == KD - 1),
                    )
                nc.scalar.activation(
                    out=hidT[:, m : m + 1], in_=ps_h[:], func=AF.Relu,
                    bias=cb1_sb[:, m : m + 1],
                )
            c2b = fin.tile([P, KF, NCLS], BF16, tag="c2b")
            load_cast(c2b[:], cw2_d[:].rearrange("(k p) c -> p k c", p=P))
            ps_l = finps.tile([1, NCLS], F32, tag="flog")
            for k2 in range(KF):
                nc.tensor.matmul(
                    ps_l[:], hidT[:, k2 : k2 + 1], c2b[:, k2, :],
                    start=(k2 == 0), stop=(k2 == KF - 1),
                )
            out_sb = fin.tile([1, NCLS], F32, tag="outsb")
            nc.vector.tensor_add(out=out_sb[:], in0=ps_l[:], in1=cb2_sb[:])
            nc.sync.dma_start(out=out_d[:], in_=out_sb[:])

    nc.finalize()
    return nc


_NC_CACHE = {}


def _get_nc(**kw):
    key = tuple(sorted(kw.items()))
    if key not in _NC_CACHE:
        _NC_CACHE[key] = build_nc(**kw)
    return _NC_CACHE[key]


def kernel(**inputs):
    """Full-model forward: takes the unsharded inputs from setup_inputs(),
    runs data-parallel across 8 NeuronCores, returns [B, NCLS] f32 logits."""
    x = np.ascontiguousarray(np.asarray(inputs["x"]), dtype=np.int32)
    mask = np.ascontiguousarray(np.asarray(inputs["attention_mask"]), dtype=np.int32)
    B = x.shape[0]
    f32 = lambda name: np.ascontiguousarray(np.asarray(inputs[name]), dtype=np.float32)
    weights = {
        name: f32(name)
        for name in (
            "tok_emb", "pos_emb", "Wq", "Wk", "Wv", "Wo", "bo",
            "ln1_w", "ln1_b", "ln2_w", "ln2_b", "W1", "b1", "W2", "b2",
            "lnf_w", "lnf_b", "cW1", "cb1", "cW2", "cb2",
        )
    }
    nc = _get_nc()
    in_maps = []
    for c in range(B):
        m = {"ids": x[c], "mask": mask[c]}
        m.update(weights)
        in_maps.append(m)
    res = run_bass_kernel_spmd(nc, in_maps, list(range(B)))
    return np.concatenate([res.results[c]["out"] for c in range(B)], axis=0)


# revision 14
# speedup vs baseline: 1.2618x; 1.0305x over previous
"""Trainium2 Bass kernel for a 4-layer GPT classifier (CMGPTClassifier).

Strategy: data-parallel over batch — each of the 8 NeuronCores runs the full
model on one sequence. All activations stay resident in SBUF in a
"layout B" = [feature-on-partitions, tokens-in-free] layout; weights stream
from HBM as casting-DMAs (f32 in DRAM -> bf16 in SBUF, software DGE);
matmuls run in bf16 with f32 PSUM accumulation.

Model (per core): S=1024 tokens, D=768, H=12 heads (HS=64), FF=3072, L=4
layers, 16 classes. h = tok_emb[x] + pos_emb; per layer:
  xn  = LN1(h);  q,k,v per head;  att = softmax(q k^T / sqrt(D)) v
  h  += concat(att) @ Wo + bo
  xn2 = LN2(h);  h += relu(xn2 @ W1 + b1) @ W2 + b2
logits = relu(LNf(h)[last] @ cW1 + cb1) @ cW2 + cb2

Implementation notes:
 - LN over the partition (feature) axis uses a ones[128,128] stationary
   matmul to produce per-token sums broadcast across partitions, so all the
   stat math runs as full-width [128, 512] vector ops.
 - Attention works in the transposed layout: scores_T[key, query] tiles, two
   heads of a pair issued back-to-back into opposite PE row-groups so they
   run concurrently; exp runs on ScalarE straight out of PSUM with the
   attention scale and key-mask bias folded in. The softmax denominator
   comes from a ones-column appended to V (lhsT [128, 65]); the att@V stage
   runs one iteration behind the scores/exp stage so the PE never stalls on
   ScalarE. Normalization multiplies by a PE-broadcast fast-reciprocal row.
"""

from contextlib import ExitStack

import numpy as np

import concourse.bacc as bacc
import concourse.bass as bass
import concourse.mybir as mybir
import concourse.tile as tile
from concourse.bass_utils import run_bass_kernel_spmd
from concourse.masks import make_identity

F32 = mybir.dt.float32
BF16 = mybir.dt.bfloat16
I32 = mybir.dt.int32
AF = mybir.ActivationFunctionType
ALU = mybir.AluOpType

P = 128


def build_nc(S=1024, L=4, H=12, D=768, FF=3072, V=32000, NCLS=16, cast_dma=True):
    HS = D // H
    KD = D // P          # 6 feature tiles
    KF = FF // P         # 24 ff tiles
    NT = S // P          # token tiles
    QBS = min(512, S)    # token block for matmul free dim
    NQ = S // QBS
    HP = H // 2          # head pairs
    SCALE = float(D) ** -0.5

    nc = bacc.Bacc("TRN2", target_bir_lowering=False)

    ids_d = nc.dram_tensor("ids", [S], I32, kind="ExternalInput")
    mask_d = nc.dram_tensor("mask", [S], I32, kind="ExternalInput")
    temb_d = nc.dram_tensor("tok_emb", [V, D], F32, kind="ExternalInput")
    pemb_d = nc.dram_tensor("pos_emb", [S, D], F32, kind="ExternalInput")
    wq_d = nc.dram_tensor("Wq", [L, H, D, HS], F32, kind="ExternalInput")
    wk_d = nc.dram_tensor("Wk", [L, H, D, HS], F32, kind="ExternalInput")
    wv_d = nc.dram_tensor("Wv", [L, H, D, HS], F32, kind="ExternalInput")
    wo_d = nc.dram_tensor("Wo", [L, D, D], F32, kind="ExternalInput")
    bo_d = nc.dram_tensor("bo", [L, D], F32, kind="ExternalInput")
    ln1w_d = nc.dram_tensor("ln1_w", [L, D], F32, kind="ExternalInput")
    ln1b_d = nc.dram_tensor("ln1_b", [L, D], F32, kind="ExternalInput")
    ln2w_d = nc.dram_tensor("ln2_w", [L, D], F32, kind="ExternalInput")
    ln2b_d = nc.dram_tensor("ln2_b", [L, D], F32, kind="ExternalInput")
    w1_d = nc.dram_tensor("W1", [L, D, FF], F32, kind="ExternalInput")
    b1_d = nc.dram_tensor("b1", [L, FF], F32, kind="ExternalInput")
    w2_d = nc.dram_tensor("W2", [L, FF, D], F32, kind="ExternalInput")
    b2_d = nc.dram_tensor("b2", [L, D], F32, kind="ExternalInput")
    lnfw_d = nc.dram_tensor("lnf_w", [D], F32, kind="ExternalInput")
    lnfb_d = nc.dram_tensor("lnf_b", [D], F32, kind="ExternalInput")
    cw1_d = nc.dram_tensor("cW1", [D, FF], F32, kind="ExternalInput")
    cb1_d = nc.dram_tensor("cb1", [FF], F32, kind="ExternalInput")
    cw2_d = nc.dram_tensor("cW2", [FF, NCLS], F32, kind="ExternalInput")
    cb2_d = nc.dram_tensor("cb2", [NCLS], F32, kind="ExternalInput")
    out_d = nc.dram_tensor("out", [1, NCLS], F32, kind="ExternalOutput")

    with tile.TileContext(nc) as tc, ExitStack() as ctx:
        consts = ctx.enter_context(tc.tile_pool(name="consts", bufs=1))
        ones_bf = consts.tile([P, P], BF16, tag="ones")
        nc.vector.memset(ones_bf[:], 1.0)
        ones_f = consts.tile([1, HS], F32, tag="onesf")
        nc.vector.memset(ones_f[:], 1.0)
        ident = consts.tile([P, P], F32, tag="ident")
        make_identity(nc, ident[:])
        eps_col = consts.tile([P, 1], F32, tag="eps")
        nc.vector.memset(eps_col[:], 1e-5)

        wstage_pool = (
            ctx.enter_context(tc.tile_pool(name="wstage", bufs=2))
            if not cast_dma else None
        )

        def load_cast(out_ap, in_ap, eng=None):
            """f32 DRAM -> bf16 SBUF; casting DMA or staged DMA + engine cast."""
            if cast_dma:
                nc.gpsimd.dma_start(out=out_ap, in_=in_ap)
                return
            np_, nf = out_ap.shape[0], out_ap.size() // out_ap.shape[0]
            st = wstage_pool.tile([np_, nf], F32, tag="wstage", name="wstage")
            shp = list(out_ap.shape)
            stv = st[:] if len(shp) == 2 else st[:].rearrange(
                "p (a b) -> p a b", a=shp[1], b=shp[2])
            nc.sync.dma_start(out=stv, in_=in_ap)
            if eng is None:
                nc.scalar.copy(out=out_ap, in_=stv)
            else:
                eng.tensor_copy(out=out_ap, in_=stv)

        ids_sb = consts.tile([P, NT], I32, tag="ids")
        nc.sync.dma_start(out=ids_sb[:], in_=ids_d[:].rearrange("(t p) -> p t", p=P))
        mask_sb = consts.tile([P, NT], I32, tag="mask")
        nc.sync.dma_start(out=mask_sb[:], in_=mask_d[:].rearrange("(t p) -> p t", p=P))
        maskf = consts.tile([P, NT], F32, tag="maskf")
        nc.vector.tensor_copy(out=maskf[:], in_=mask_sb[:])
        mbias = consts.tile([P, NT], F32, tag="mbias")
        nc.vector.tensor_scalar(
            out=mbias[:], in0=maskf[:], scalar1=1.0, scalar2=30.0,
            op0=ALU.subtract, op1=ALU.mult,
        )

        # small per-layer params as per-partition column banks
        def col_bank(tag, dram, inner, pat):
            t = consts.tile([P, L, inner] if pat == "l" else [P, inner], F32, tag=tag, name=tag)
            if pat == "l":
                nc.sync.dma_start(out=t[:], in_=dram[:].rearrange("l (k p) -> p l k", p=P))
            else:
                nc.sync.dma_start(out=t[:], in_=dram[:].rearrange("(k p) -> p k", p=P))
            return t

        ln1w = col_bank("ln1w", ln1w_d, KD, "l")
        ln1b = col_bank("ln1b", ln1b_d, KD, "l")
        ln2w = col_bank("ln2w", ln2w_d, KD, "l")
        ln2b = col_bank("ln2b", ln2b_d, KD, "l")
        bo_sb = col_bank("bo", bo_d, KD, "l")
        b2_sb = col_bank("b2", b2_d, KD, "l")
        b1_sb = col_bank("b1", b1_d, KF, "l")
        lnfw = col_bank("lnfw", lnfw_d, KD, "f")
        lnfb = col_bank("lnfb", lnfb_d, KD, "f")
        cb1_sb = col_bank("cb1", cb1_d, KF, "f")
        cb2_sb = consts.tile([1, NCLS], F32, tag="cb2")
        nc.sync.dma_start(out=cb2_sb[:], in_=cb2_d[None, :])

        # residual stream + post-LN activations, persistent
        h_pool = ctx.enter_context(tc.tile_pool(name="h", bufs=1))
        h_B = [h_pool.tile([P, S], F32, tag=f"h{k}", name=f"h{k}") for k in range(KD)]
        xn_pool = ctx.enter_context(tc.tile_pool(name="xn", bufs=1))
        xn = [xn_pool.tile([P, S], BF16, tag=f"xn{k}", name=f"xn{k}") for k in range(KD)]

        # ---------------- embedding ----------------
        with tc.tile_pool(name="emb", bufs=3) as emb, tc.tile_pool(
            name="emb_ps", bufs=2, space="PSUM"
        ) as emb_ps:
            for t in range(NT):
                gat = emb.tile([P, D], F32, tag="gat")
                nc.gpsimd.indirect_dma_start(
                    out=gat[:],
                    out_offset=None,
                    in_=temb_d[:],
                    in_offset=bass.IndirectOffsetOnAxis(ap=ids_sb[:, t : t + 1], axis=0),
                )
                pos = emb.tile([P, D], F32, tag="pos")
                nc.sync.dma_start(out=pos[:], in_=pemb_d[t * P : (t + 1) * P, :])
                ha = emb.tile([P, D], F32, tag="ha")
                nc.vector.tensor_add(out=ha[:], in0=gat[:], in1=pos[:])
                for k in range(KD):
                    pst = emb_ps.tile([P, P], F32, tag="pst")
                    nc.tensor.transpose(
                        out=pst[:], in_=ha[:, k * P : (k + 1) * P], identity=ident[:]
                    )
                    nc.vector.tensor_copy(
                        out=h_B[k][:, t * P : (t + 1) * P], in_=pst[:]
                    )

        # ---------------- layernorm helper ----------------
        def layernorm(li, w_bank, b_bank, dst):
            """dst[k] (bf16) = LN(h_B) * w + b; feature axis = partitions."""
            with tc.tile_pool(name=f"ln{li}", bufs=2) as lnp, tc.tile_pool(
                name=f"ln{li}s", bufs=2
            ) as lns, tc.tile_pool(name=f"ln{li}_ps", bufs=2, space="PSUM") as psp:
                for qb in range(NQ):
                    qs = slice(qb * QBS, (qb + 1) * QBS)
                    ps_sum = psp.tile([P, QBS], F32, tag="pssum")
                    ps_sq = psp.tile([P, QBS], F32, tag="pssq")
                    for k in range(KD):
                        hb = lnp.tile([P, QBS], BF16, tag="hb")
                        nc.vector.tensor_copy(out=hb[:], in_=h_B[k][:, qs])
                        nc.tensor.matmul(
                            ps_sum[:], ones_bf[:], hb[:],
                            start=(k == 0), stop=(k == KD - 1),
                        )
                        sq = lnp.tile([P, QBS], BF16, tag="sq")
                        nc.vector.tensor_mul(out=sq[:], in0=hb[:], in1=hb[:])
                        nc.tensor.matmul(
                            ps_sq[:], ones_bf[:], sq[:],
                            start=(k == 0), stop=(k == KD - 1),
                        )
                    mean = lns.tile([P, QBS], F32, tag="mean")
                    nc.vector.tensor_scalar_mul(out=mean[:], in0=ps_sum[:], scalar1=1.0 / D)
                    msq = lnp.tile([P, QBS], F32, tag="msq")
                    nc.vector.tensor_scalar_mul(out=msq[:], in0=ps_sq[:], scalar1=1.0 / D)
                    var = lnp.tile([P, QBS], F32, tag="var")
                    nc.vector.tensor_mul(out=var[:], in0=mean[:], in1=mean[:])
                    nc.vector.tensor_sub(out=var[:], in0=msq[:], in1=var[:])
                    std = lnp.tile([P, QBS], F32, tag="std")
                    nc.scalar.activation(out=std[:], in_=var[:], func=AF.Sqrt, bias=eps_col[:])
                    rstd = lns.tile([P, QBS], F32, tag="rstd")
                    nc.vector.reciprocal_approx_fast(out=rstd[:], in_=std[:])
                    for k in range(KD):
                        tmp = lnp.tile([P, QBS], F32, tag="tmp")
                        nc.vector.tensor_sub(out=tmp[:], in0=h_B[k][:, qs], in1=mean[:])
                        nc.vector.tensor_mul(out=dst[k][:, qs], in0=tmp[:], in1=rstd[:])
                        nc.vector.tensor_scalar(
                            out=dst[k][:, qs], in0=dst[k][:, qs],
                            scalar1=w_bank[:, k : k + 1], scalar2=b_bank[:, k : k + 1],
                            op0=ALU.mult, op1=ALU.add,
                        )

        # ---------------- layers ----------------
        for l in range(L):
            layernorm(f"1_{l}", ln1w[:, l, :], ln1b[:, l, :], xn)

            with ExitStack() as lctx:
                qkvw = lctx.enter_context(tc.tile_pool(name=f"qkvw{l}", bufs=1))
                qkp = lctx.enter_context(tc.tile_pool(name=f"qk{l}", bufs=1))
                vp = lctx.enter_context(tc.tile_pool(name=f"v{l}", bufs=1))
                attop = lctx.enter_context(tc.tile_pool(name=f"atto{l}", bufs=1))
                sst = lctx.enter_context(tc.tile_pool(name=f"sst{l}", bufs=3))

                # load qkv weights (casting DMA: f32 DRAM -> bf16 SBUF)
                w_b = {}
                for name, dram in (("q", wq_d), ("k", wk_d), ("v", wv_d)):
                    wb = qkvw.tile([P, KD, H * HS], BF16, tag=f"w{name}b", name=f"w{name}b")
                    w_b[name] = wb
                    for k in range(KD):
                        load_cast(
                            wb[:, k, :].rearrange("p (h e) -> p h e", e=HS),
                            dram[l][:, k * P : (k + 1) * P, :].rearrange("h p e -> p h e"),
                        )

                # V for all heads/token-tiles first (own psum scope)
                v_all = [vp.tile([P, H, HS + 1], BF16, tag=f"v{t}", name=f"v{t}") for t in range(NT)]
                with tc.tile_pool(name=f"v_ps{l}", bufs=3, space="PSUM") as vps:
                    wvb = w_b["v"]
                    nsplits = []
                    off = 0
                    while off < H * HS:
                        nsz = min(512, H * HS - off)
                        nsplits.append((off, nsz))
                        off += nsz
                    for t in range(NT):
                        nc.vector.memset(v_all[t][:, :, HS : HS + 1], 1.0)
                        for noff, nsz in nsplits:
                            ps = vps.tile([P, 512], F32, tag="psv")
                            for k in range(KD):
                                nc.tensor.matmul(
                                    ps[:, :nsz],
                                    xn[k][:, t * P : (t + 1) * P],
                                    wvb[:, k, noff : noff + nsz],
                                    start=(k == 0), stop=(k == KD - 1),
                                )
                            h0 = noff // HS
                            nh = nsz // HS
                            nc.scalar.copy(
                                out=v_all[t][:, h0 : h0 + nh, 0:HS],
                                in_=ps[:, :nsz].rearrange("p (h e) -> p h e", e=HS),
                            )

                # pipelined: qkv(pair+1) | scores/exp(pair) | att@V+norm(pair-1)
                q_pair = [qkp.tile([P, S], BF16, tag=f"q{i}", name=f"q{i}") for i in range(HP)]
                k_pair = [qkp.tile([P, S], BF16, tag=f"k{i}", name=f"k{i}") for i in range(HP)]
                atto = [attop.tile([P, S], BF16, tag=f"ao{i}", name=f"ao{i}") for i in range(HP)]

                with tc.tile_pool(name=f"att{l}", bufs=48) as attp, tc.tile_pool(
                    name=f"attsm{l}", bufs=3
                ) as attsm, tc.tile_pool(
                    name=f"qk_ps{l}", bufs=2, space="PSUM"
                ) as qkps, tc.tile_pool(
                    name=f"att_pss{l}", bufs=2, space="PSUM"
                ) as attps, tc.tile_pool(
                    name=f"att_psav{l}", bufs=2, space="PSUM"
                ) as attps2:

                    def qkv_pair(i):
                        for name, dest in (("q", q_pair), ("k", k_pair)):
                            wb = w_b[name]
                            for qb in range(NQ):
                                qs = slice(qb * QBS, (qb + 1) * QBS)
                                ps = qkps.tile([P, QBS], F32, tag="psqk", name="psqk")
                                for k in range(KD):
                                    st0, sp0 = (k == 0), (k == KD - 1)
                                    nc.tensor.matmul(
                                        ps[0:HS, :],
                                        wb[:, k, (2 * i) * HS : (2 * i + 1) * HS],
                                        xn[k][:, qs],
                                        start=st0, stop=sp0,
                                        tile_position=(0, 0),
                                        skip_group_check=True,
                                    )
                                    nc.tensor.matmul(
                                        ps[HS : 2 * HS, :],
                                        wb[:, k, (2 * i + 1) * HS : (2 * i + 2) * HS],
                                        xn[k][:, qs],
                                        start=st0, stop=sp0,
                                        tile_position=(0, HS),
                                        skip_group_check=True,
                                    )
                                nc.vector.tensor_copy(out=dest[i][:, qs], in_=ps[:])

                    def scores_exp(pi):
                        ats = {}
                        for qb in range(NQ):
                            qs = slice(qb * QBS, (qb + 1) * QBS)
                            for ho in (0, 1):
                                r0 = ho * HS
                                lst = []
                                for kt in range(NT):
                                    ps_s = attps.tile([P, QBS], F32, tag=f"pss{ho}", name=f"pss{ho}")
                                    nc.tensor.matmul(
                                        ps_s[:],
                                        k_pair[pi][r0 : r0 + HS, kt * P : (kt + 1) * P],
                                        q_pair[pi][r0 : r0 + HS, qs],
                                        start=True, stop=True,
                                    )
                                    at = attp.tile([P, QBS], BF16, tag="attT", name="attT")
                                    nc.scalar.activation(
                                        out=at[:], in_=ps_s[:], func=AF.Exp,
                                        bias=mbias[:, kt : kt + 1], scale=SCALE,
                                    )
                                    lst.append(at)
                                ats[(qb, ho)] = lst
                        return ats

                    def av_norm(pi, ats):
                        for qb in range(NQ):
                            qs = slice(qb * QBS, (qb + 1) * QBS)
                            ps_av = []
                            for ho in (0, 1):
                                hd = 2 * pi + ho
                                ps = attps2.tile([P, QBS], F32, tag="psav", name="psav")
                                ps_av.append(ps)
                                for kt in range(NT):
                                    nc.tensor.matmul(
                                        ps[0 : HS + 1, :],
                                        v_all[kt][:, hd, :],
                                        ats[(qb, ho)][kt][:],
                                        start=(kt == 0), stop=(kt == NT - 1),
                                    )
                            ps_bc = attps.tile([P, QBS], F32, tag="pss0", name="pss0")
                            for ho in (0, 1):
                                den = attsm.tile([1, QBS], F32, tag=f"den{ho}", name=f"den{ho}")
                                nc.scalar.copy(out=den[:], in_=ps_av[ho][HS : HS + 1, :])
                                denr = attsm.tile([1, QBS], F32, tag=f"denr{ho}", name=f"denr{ho}")
                                nc.vector.reciprocal_approx_fast(out=denr[:], in_=den[:])
                                denb = attsm.tile([1, QBS], BF16, tag=f"denb{ho}", name=f"denb{ho}")
                                nc.vector.tensor_copy(out=denb[:], in_=denr[:])
                                nc.tensor.matmul(
                                    ps_bc[ho * HS : (ho + 1) * HS, :],
                                    ones_bf[0:1, 0:HS], denb[:],
                                    start=True, stop=True,
                                    tile_position=(0, ho * HS),
                                    skip_group_check=True,
                                )
                            rb = attsm.tile([P, QBS], BF16, tag="rb", name="rb")
                            nc.scalar.copy(out=rb[:], in_=ps_bc[:])
                            for ho in (0, 1):
                                r0 = ho * HS
                                nc.vector.tensor_mul(
                                    out=atto[pi][r0 : r0 + HS, qs],
                                    in0=ps_av[ho][0:HS, :], in1=rb[r0 : r0 + HS, :],
                                )

                    qkv_pair(0)
                    pend = None
                    for pi in range(HP):
                        if pi + 1 < HP:
                            qkv_pair(pi + 1)
                        ats = scores_exp(pi)
                        if pend is not None:
                            av_norm(*pend)
                        pend = (pi, ats)
                    av_norm(*pend)

                # --- output projection + residual ---
                with tc.tile_pool(name=f"wo{l}", bufs=1) as wop, tc.tile_pool(
                    name=f"wo_ps{l}", bufs=3, space="PSUM"
                ) as wops:
                    wob = wop.tile([P, KD, D], BF16, tag="wob")
                    load_cast(
                        wob[:],
                        wo_d[l][:, :].rearrange("(k p) m -> p k m", p=P),
                    )
                    for do in range(KD):
                        for qb in range(NQ):
                            qs = slice(qb * QBS, (qb + 1) * QBS)
                            ps = wops.tile([P, QBS], F32, tag="pswo")
                            for di in range(KD):
                                nc.tensor.matmul(
                                    ps[:],
                                    wob[:, di, do * P : (do + 1) * P],
                                    atto[di][:, qs],
                                    start=(di == 0), stop=(di == KD - 1),
                                )
                            tt = sst.tile([P, QBS], F32, tag="two")
                            nc.scalar.activation(
                                out=tt[:], in_=ps[:], func=AF.Identity,
                                bias=bo_sb[:, l, do : do + 1],
                            )
                            nc.vector.tensor_add(
                                out=h_B[do][:, qs], in0=h_B[do][:, qs], in1=tt[:]
                            )

            layernorm(f"2_{l}", ln2w[:, l, :], ln2b[:, l, :], xn)

            # --- MLP ---
            with tc.tile_pool(name=f"w1p{l}", bufs=1) as w1p, tc.tile_pool(
                name=f"mst{l}", bufs=3
            ) as mst, tc.tile_pool(name=f"ffp{l}", bufs=1) as ffp, tc.tile_pool(
                name=f"w2p{l}", bufs=4
            ) as w2p, tc.tile_pool(
                name=f"mlp_ps{l}", bufs=2, space="PSUM"
            ) as mlps, tc.tile_pool(
                name=f"mlp_ps2{l}", bufs=1, space="PSUM"
            ) as mlps2:
                w1b = [w1p.tile([P, FF], BF16, tag=f"w1b{k}", name=f"w1b{k}") for k in range(KD)]
                for k in range(KD):
                    load_cast(w1b[k][:], w1_d[l][k * P : (k + 1) * P, :], eng=nc.gpsimd)
                ff = [ffp.tile([P, QBS], BF16, tag=f"ff{m}", name=f"ff{m}") for m in range(KF)]
                for qb in range(NQ):
                    qs = slice(qb * QBS, (qb + 1) * QBS)
                    for m in range(KF):
                        ps = mlps.tile([P, QBS], F32, tag="psw1")
                        for k in range(KD):
                            nc.tensor.matmul(
                                ps[:],
                                w1b[k][:, m * P : (m + 1) * P],
                                xn[k][:, qs],
                                start=(k == 0), stop=(k == KD - 1),
                            )
                        # relu(x + b1) on DVE: add bias column, clamp at 0
                        nc.vector.tensor_scalar(
                            out=ff[m][:], in0=ps[:],
                            scalar1=b1_sb[:, l, m : m + 1], scalar2=0.0,
                            op0=ALU.add, op1=ALU.max,
                        )
                    # W2: k2-outer accumulation into KD open psums
                    ps_o = [mlps2.tile([P, QBS], F32, tag=f"psw2_{do}", name=f"psw2_{do}") for do in range(KD)]
                    for k2 in range(KF):
                        w2b = w2p.tile([P, D], BF16, tag="w2b")
                        load_cast(w2b[:], w2_d[l][k2 * P : (k2 + 1) * P, :], eng=nc.gpsimd)
                        for do in range(KD):
                            nc.tensor.matmul(
                                ps_o[do][:],
                                w2b[:, do * P : (do + 1) * P],
                                ff[k2][:],
                                start=(k2 == 0), stop=(k2 == KF - 1),
                            )
                    for do in range(KD):
                        tt = mst.tile([P, QBS], F32, tag="tw2")
                        nc.scalar.activation(
                            out=tt[:], in_=ps_o[do][:], func=AF.Identity,
                            bias=b2_sb[:, l, do : do + 1],
                        )
                        nc.vector.tensor_add(
                            out=h_B[do][:, qs], in0=h_B[do][:, qs], in1=tt[:]
                        )

        # ---------------- final LN (last token) + classifier ----------------
        with tc.tile_pool(name="fin", bufs=1) as fin, tc.tile_pool(
            name="finst", bufs=3
        ) as finst, tc.tile_pool(name="fin_ps", bufs=1, space="PSUM") as finps:
            hcb = fin.tile([P, KD], BF16, tag="hcb")
            sqc = fin.tile([P, KD], BF16, tag="sqc")
            for k in range(KD):
                nc.vector.tensor_copy(out=hcb[:, k : k + 1], in_=h_B[k][:, S - 1 : S])
                nc.vector.tensor_mul(
                    out=sqc[:, k : k + 1],
                    in0=h_B[k][:, S - 1 : S], in1=h_B[k][:, S - 1 : S],
                )
            ps_sum = finps.tile([P, 1], F32, tag="fsum")
            ps_sq = finps.tile([P, 1], F32, tag="fsq")
            for k in range(KD):
                nc.tensor.matmul(
                    ps_sum[:], ones_bf[:], hcb[:, k : k + 1],
                    start=(k == 0), stop=(k == KD - 1),
                )
                nc.tensor.matmul(
                    ps_sq[:], ones_bf[:], sqc[:, k : k + 1],
                    start=(k == 0), stop=(k == KD - 1),
                )
            mean = fin.tile([P, 1], F32, tag="fmean")
            nc.vector.tensor_scalar_mul(out=mean[:], in0=ps_sum[:], scalar1=1.0 / D)
            msq = fin.tile([P, 1], F32, tag="fmsq")
            nc.vector.tensor_scalar_mul(out=msq[:], in0=ps_sq[:], scalar1=1.0 / D)
            var = fin.tile([P, 1], F32, tag="fvar")
            nc.vector.tensor_mul(out=var[:], in0=mean[:], in1=mean[:])
            nc.vector.tensor_sub(out=var[:], in0=msq[:], in1=var[:])
            std = fin.tile([P, 1], F32, tag="fstd")
            nc.scalar.activation(out=std[:], in_=var[:], func=AF.Sqrt, bias=eps_col[:])
            rstd = fin.tile([P, 1], F32, tag="frstd")
            nc.vector.reciprocal_approx_fast(out=rstd[:], in_=std[:])
            xnl = fin.tile([P, KD], BF16, tag="xnl")
            for k in range(KD):
                tmp = finst.tile([P, 1], F32, tag="ftmp")
                nc.vector.tensor_sub(out=tmp[:], in0=h_B[k][:, S - 1 : S], in1=mean[:])
                nc.vector.tensor_mul(out=xnl[:, k : k + 1], in0=tmp[:], in1=rstd[:])
                nc.vector.tensor_scalar(
                    out=xnl[:, k : k + 1], in0=xnl[:, k : k + 1],
                    scalar1=lnfw[:, k : k + 1], scalar2=lnfb[:, k : k + 1],
                    op0=ALU.mult, op1=ALU.add,
                )
            # classifier mlp
            c1b = [fin.tile([P, FF], BF16, tag=f"c1b{k}", name=f"c1b{k}") for k in range(KD)]
            for k in range(KD):
                load_cast(c1b[k][:], cw1_d[k * P : (k + 1) * P, :], eng=nc.gpsimd)
            hidT = fin.tile([P, KF], BF16, tag="hidT")
            for m in range(KF):
                ps_h = finps.tile([P, 1], F32, tag="fh")
                for k in range(KD):
                    nc.tensor.matmul(
                        ps_h[:], c1b[k][:, m * P : (m + 1) * P], xnl[:, k : k + 1],
                        start=(k == 0), stop=(k == KD - 1),
                    )
                nc.scalar.activation(
                    out=hidT[:, m : m + 1], in_=ps_h[:], func=AF.Relu,
                    bias=cb1_sb[:, m : m + 1],
                )
            c2b = fin.tile([P, KF, NCLS], BF16, tag="c2b")
            load_cast(c2b[:], cw2_d[:].rearrange("(k p) c -> p k c", p=P))
            ps_l = finps.tile([1, NCLS], F32, tag="flog")
            for k2 in range(KF):
                nc.tensor.matmul(
                    ps_l[:], hidT[:, k2 : k2 + 1], c2b[:, k2, :],
                    start=(k2 == 0), stop=(k2 == KF - 1),
                )
            out_sb = fin.tile([1, NCLS], F32, tag="outsb")
            nc.vector.tensor_add(out=out_sb[:], in0=ps_l[:], in1=cb2_sb[:])
            nc.sync.dma_start(out=out_d[:], in_=out_sb[:])

    nc.finalize()
    return nc


_NC_CACHE = {}


def _get_nc(**kw):
    key = tuple(sorted(kw.items()))
    if key not in _NC_CACHE:
        _NC_CACHE[key] = build_nc(**kw)
    return _NC_CACHE[key]


def kernel(**inputs):
    """Full-model forward: takes the unsharded inputs from setup_inputs(),
    runs data-parallel across 8 NeuronCores, returns [B, NCLS] f32 logits."""
    x = np.ascontiguousarray(np.asarray(inputs["x"]), dtype=np.int32)
    mask = np.ascontiguousarray(np.asarray(inputs["attention_mask"]), dtype=np.int32)
    B = x.shape[0]
    f32 = lambda name: np.ascontiguousarray(np.asarray(inputs[name]), dtype=np.float32)
    weights = {
        name: f32(name)
        for name in (
            "tok_emb", "pos_emb", "Wq", "Wk", "Wv", "Wo", "bo",
            "ln1_w", "ln1_b", "ln2_w", "ln2_b", "W1", "b1", "W2", "b2",
            "lnf_w", "lnf_b", "cW1", "cb1", "cW2", "cb2",
        )
    }
    nc = _get_nc()
    in_maps = []
    for c in range(B):
        m = {"ids": x[c], "mask": mask[c]}
        m.update(weights)
        in_maps.append(m)
    res = run_bass_kernel_spmd(nc, in_maps, list(range(B)))
    return np.concatenate([res.results[c]["out"] for c in range(B)], axis=0)
